# revision 2
# baseline (speedup 1.0000x reference)
import sys

if "/opt/trn_rl_repo" not in sys.path:
    sys.path.insert(0, "/opt/trn_rl_repo")

import numpy as np
import ml_dtypes

from concourse import bass, tile, bacc
from concourse.bass import mybir

F32 = mybir.dt.float32
BF16 = mybir.dt.bfloat16

N_CORES = 8
N_TOTAL = 32768
N_CORE = N_TOTAL // N_CORES  # 4096 rows per core
D = 1024
C = 64
K = 16
DEPTH = 4
M = 1024
N_STAGES = 2
ROWS_STAGE = N_CORE // N_STAGES  # 2048
ALU = mybir.AluOpType
AFT = mybir.ActivationFunctionType


def build_program(dims, repeat=1):
    """dims: python list of 256 ints (compile-time gather indices)."""
    nc = bacc.Bacc()
    x_d = nc.declare_dram_parameter("x", [N_CORE, D], F32, isOutput=False)
    thr_d = nc.declare_dram_parameter("thrcols", [128, 15], F32, isOutput=False)
    lut_d = nc.declare_dram_parameter("lutT", [C * K, M], BF16, isOutput=False)
    kvec_d = nc.declare_dram_parameter("kvec", [128, 1], F32, isOutput=False)
    out_d = nc.declare_dram_parameter("out", [N_CORE, M], F32, isOutput=True)

    with tile.TileContext(nc) as tc:
        from contextlib import ExitStack
        es = ExitStack()
        pers = es.enter_context(tc.tile_pool(name="pers", bufs=1))

        def ptile(shape, dtype, name):
            return pers.tile(shape, dtype, name=name, tag=name)

        # ---- persistent tiles ----
        lutT = ptile([128, 8, M], BF16, "lutT_sb")       # [ck%128, j, m]
        thr = ptile([128, 15], F32, "thr_sb")
        kvec = ptile([128, 1], F32, "kvec_sb")
        ET = ptile([128, N_STAGES * 8, 2048], BF16, "ET_sb")
        xTS = ptile([128, 8, 32, 32], F32, "xTS_sb")     # [p, tau, u_hi, s]
        chosenT = ptile([128, DEPTH, 1024], F32, "chosenT_sb")
        bucketbf = ptile([128, 1024], BF16, "bucketbf_sb")
        tmps = [ptile([128, 1024], F32, f"tmp{ti}_sb") for ti in range(10)]
        b0, b1, b2, tA, tB, tC, tD, tE, tF, tG = tmps
        I8 = mybir.dt.int8
        b0i = ptile([128, 1024], I8, "b0i_sb")
        b1i = ptile([128, 1024], I8, "b1i_sb")

        xpool = es.enter_context(tc.tile_pool(name="xpool", bufs=2))
        opool = es.enter_context(tc.tile_pool(name="opool", bufs=2))
        pspool = es.enter_context(
            tc.tile_pool(name="pspool", bufs=2, space=bass.MemorySpace.PSUM)
        )

        nc.sync.dma_start(thr[:], thr_d[:])
        nc.sync.dma_start(kvec[:], kvec_d[:])
        for j in range(8):
            nc.sync.dma_start(lutT[:, j, :], lut_d[j * 128:(j + 1) * 128, :])

        # thr column APs
        def tcol(i):
            return thr[:, i:i + 1]

        for s in [s for _ in range(repeat) for s in range(N_STAGES)]:
            for hp in range(2):  # which 1024-row half of the stage
                # load + stream-transpose 8 x-tiles of this half
                for tau in range(8):
                    r0 = s * ROWS_STAGE + hp * 1024 + tau * 128
                    xt = xpool.tile([128, D], F32, name="xt", tag="xt")
                    nc.sync.dma_start(xt[:], x_d[r0:r0 + 128, :])
                    nc.vector.transpose(
                        xTS[:, tau].rearrange("p a b -> p (a b)"), xt[:]
                    )
                # row-copy DMAs: for each (c, d) pull column u into chosenT
                for c in range(C):
                    P = hp * 64 + c
                    for d in range(DEPTH):
                        u = dims[c * DEPTH + d]
                        src = xTS[u % 32::32, :, u // 32, :]          # [4, 8, 32]
                        dst = chosenT[P:P + 1, d, :].rearrange(
                            "p (bi t s) -> p bi t s", bi=4, t=8, s=32
                        )
                        nc.sync.dma_start(dst, src)

                # ---- descent on [128=(hp,c) x 1024] ----  (hp covers only one
                # half of partitions with fresh data per hp iteration; compute
                # once per stage after both halves are gathered)
            xd = [chosenT[:, d, :] for d in range(DEPTH)]
            nc.vector.tensor_scalar(b0[:], xd[0], tcol(0), None, ALU.is_gt)
            nc.vector.tensor_copy(b0i[:], b0[:])
            nc.vector.tensor_scalar(tA[:], b0[:], tcol(2), tcol(1), ALU.mult, ALU.add)
            nc.vector.tensor_tensor(b1[:], xd[1], tA[:], ALU.is_gt)
            nc.vector.tensor_copy(b1i[:], b1[:])

            nc.vector.tensor_scalar(tB[:], b1[:], tcol(4), tcol(3), ALU.mult, ALU.add)
            nc.vector.tensor_scalar(tC[:], b1[:], tcol(6), tcol(5), ALU.mult, ALU.add)
            nc.vector.tensor_copy(tA[:], tB[:])
            nc.vector.copy_predicated(tA[:], b0i[:], tC[:])
            nc.vector.tensor_tensor(b2[:], xd[2], tA[:], ALU.is_gt)

            nc.vector.tensor_scalar(tB[:], b2[:], tcol(8), tcol(7), ALU.mult, ALU.add)
            nc.vector.tensor_scalar(tC[:], b2[:], tcol(10), tcol(9), ALU.mult, ALU.add)
            nc.vector.tensor_scalar(tD[:], b2[:], tcol(12), tcol(11), ALU.mult, ALU.add)
            nc.vector.tensor_scalar(tE[:], b2[:], tcol(14), tcol(13), ALU.mult, ALU.add)
            nc.vector.tensor_copy(tF[:], tB[:])
            nc.vector.copy_predicated(tF[:], b1i[:], tC[:])
            nc.vector.tensor_copy(tG[:], tD[:])
            nc.vector.copy_predicated(tG[:], b1i[:], tE[:])
            nc.vector.tensor_copy(tA[:], tF[:])
            nc.vector.copy_predicated(tA[:], b0i[:], tG[:])
            nc.vector.tensor_tensor(tD[:], xd[3], tA[:], ALU.is_gt)   # b3 -> tD

            # bucket = 8*b0 + 4*b1 + 2*b2 + b3   (built as ((b0*2+b1)*2+b2)*2+b3)
            nc.vector.scalar_tensor_tensor(tB[:], b0[:], 2.0, b1[:], ALU.mult, ALU.add)
            nc.vector.scalar_tensor_tensor(tC[:], tB[:], 2.0, b2[:], ALU.mult, ALU.add)
            nc.vector.scalar_tensor_tensor(
                bucketbf[:], tC[:], 2.0, tD[:], ALU.mult, ALU.add
            )

            # ---- E^T: replicate bucket row to 8 partitions per k, compare ----
            for j in range(8):
                col = s * 8 + j
                for hp in range(2):
                    fsl = slice(hp * 1024, (hp + 1) * 1024)
                    nc.scalar.dma_start(
                        ET[0:8, col, fsl],
                        bucketbf[hp * 64 + 8 * j:hp * 64 + 8 * j + 8, :],
                    )
                    for dbl in range(4):
                        w = 8 << dbl
                        nc.scalar.dma_start(
                            ET[w:2 * w, col, fsl], ET[0:w, col, fsl]
                        )
                nc.vector.tensor_scalar(
                    ET[:, col, :], ET[:, col, :], kvec[:], None,
                    ALU.is_equal,
                )

            # ---- matmul + output ----
            for i in range(16):
                ps = [
                    pspool.tile([128, 512], F32, name=f"ps{mc}", tag=f"ps{mc}")
                    for mc in range(2)
                ]
                for j in range(8):
                    lhsT = ET[:, s * 8 + j, i * 128:(i + 1) * 128]
                    for mc in range(2):
                        nc.tensor.matmul(
                            ps[mc][:], lhsT, lutT[:, j, mc * 512:(mc + 1) * 512],
                            start=(j == 0), stop=(j == 7),
                        )
                osb = opool.tile([128, M], F32, name="osb", tag="osb")
                nc.scalar.activation(osb[:, 0:512], ps[0][:], AFT.Copy)
                nc.scalar.activation(osb[:, 512:1024], ps[1][:], AFT.Copy)

                ih = i % 8
                hp = i // 8
                base = s * ROWS_STAGE + hp * 1024 + (ih % 2) * 512 + (ih // 2) * 32
                dview = out_d[:].rearrange("(a b c) m -> a b c m", b=4, c=32)
                a0 = base // 128
                nc.sync.dma_start(dview[a0:a0 + 4, (base % 128) // 32, :, :], osb[:])
        es.close()
    nc.finalize()
    return nc


def _prep_inputs(inputMatrix, dims, thresholds, lut):
    x = np.ascontiguousarray(np.asarray(inputMatrix, dtype=np.float32))
    dims = [int(v) for v in np.asarray(dims).ravel()]
    thr = np.asarray(thresholds, dtype=np.float32).reshape(C, K - 1)
    lut = np.asarray(lut, dtype=np.float32)

    # thrcols [128, 15]: t0,t1,d21,t3,d43,t5,d65,t7,d87,t9,d109,t11,d1211,t13,d1413
    tcols = np.empty((C, 15), dtype=np.float32)
    tcols[:, 0] = thr[:, 0]
    pairs = [(1, 2), (3, 4), (5, 6), (7, 8), (9, 10), (11, 12), (13, 14)]
    for idx, (lo, hi) in enumerate(pairs):
        tcols[:, 1 + 2 * idx] = thr[:, lo]
        tcols[:, 2 + 2 * idx] = thr[:, hi] - thr[:, lo]
    thrcols = np.concatenate([tcols, tcols], axis=0)  # [128, 15]

    # lutT [j*128 + k*8 + c_loc, m] = lut[m, 8j + c_loc, k]
    lt = lut.reshape(M, 8, 8, K).transpose(1, 3, 2, 0).reshape(C * K, M)
    lutT = lt.astype(ml_dtypes.bfloat16)

    kvec = (np.arange(128) // 8).astype(np.float32).reshape(128, 1)
    return x, dims, thrcols, lutT, kvec


def _prep_all(inputs):
    x, dims_l, thrcols, lutT, kvec = _prep_inputs(
        inputs["inputMatrix"], inputs["dims"], inputs["thresholds"], inputs["lut"]
    )
    nc = build_program(dims_l)
    in_maps = [
        {
            "x": np.ascontiguousarray(x[i * N_CORE:(i + 1) * N_CORE]),
            "thrcols": thrcols,
            "lutT": lutT,
            "kvec": kvec,
        }
        for i in range(N_CORES)
    ]
    return nc, in_maps


def kernel(inputMatrix, dims, thresholds, lut, selection_matrix=None,
           tree_des_mat=None):
    from concourse.bass_utils import run_bass_kernel_spmd

    nc, in_maps = _prep_all(
        {
            "inputMatrix": inputMatrix,
            "dims": dims,
            "thresholds": thresholds,
            "lut": lut,
        }
    )
    res = run_bass_kernel_spmd(nc, in_maps, list(range(N_CORES)))
    out = np.concatenate(
        [np.asarray(res.results[i]["out"]) for i in range(N_CORES)], axis=0
    )
    return out.astype(np.float32)



# revision 4
# speedup vs baseline: 1.1742x; 1.1742x over previous
import sys

if "/opt/trn_rl_repo" not in sys.path:
    sys.path.insert(0, "/opt/trn_rl_repo")

import numpy as np
import ml_dtypes

from concourse import bass, tile, bacc
from concourse.bass import mybir

F32 = mybir.dt.float32
BF16 = mybir.dt.bfloat16
I8 = mybir.dt.int8

N_CORES = 8
N_TOTAL = 32768
N_CORE = N_TOTAL // N_CORES  # 4096 rows per core
D = 1024
C = 64
K = 16
DEPTH = 4
M = 1024
NT = N_CORE // 128           # 32 tiles of 128 rows
SIZES = [2, 6, 8, 8, 8]      # tiles per group (even, for XBAR alignment)
G = 8
NG = len(SIZES)
ALU = mybir.AluOpType
AFT = mybir.ActivationFunctionType

NCONST = 22  # 15 thresholds + 7 pair diffs, per (h,c) partition


def build_program():
    nc = bacc.Bacc()
    # chosenT, group-blocked: per group [128=(h,c), d, Gg*64] sections
    x_d = nc.declare_dram_parameter("cht", [128, 2 * NT * 128], F32, isOutput=False)
    cst_d = nc.declare_dram_parameter("cst", [128, NCONST], F32, isOutput=False)
    lut_d = nc.declare_dram_parameter("lutrj", [128, 9, M], BF16, isOutput=False)
    out_d = nc.declare_dram_parameter("out", [N_CORE, M], BF16, isOutput=True)

    with tile.TileContext(nc) as tc:
        from contextlib import ExitStack
        es = ExitStack()
        pers = es.enter_context(tc.tile_pool(name="pers", bufs=1))

        lutrj = pers.tile([128, 9, M], BF16, name="lutrj_sb", tag="lutrj_sb")
        cst = pers.tile([128, NCONST], F32, name="cst_sb", tag="cst_sb")

        lutT = lutrj
        kpat = lutrj[:, 8, :]

        def tcol(i):
            return cst[:, i:i + 1]

        chpool = es.enter_context(tc.tile_pool(name="chpool", bufs=3))
        bkpool = es.enter_context(tc.tile_pool(name="bkpool", bufs=5))
        bnpool = es.enter_context(tc.tile_pool(name="bnpool", bufs=5))
        tmppool = es.enter_context(tc.tile_pool(name="tmppool", bufs=2))
        epool = es.enter_context(tc.tile_pool(name="epool", bufs=6))
        etpool = es.enter_context(tc.tile_pool(name="etpool", bufs=8))
        opool = es.enter_context(tc.tile_pool(name="opool", bufs=2))
        pspool = es.enter_context(
            tc.tile_pool(name="pspool", bufs=4, space=bass.MemorySpace.PSUM)
        )

        nc.sync.dma_start(cst[:], cst_d[:])
        nc.sync.dma_start(lutrj[:, 8:9, :], lut_d[:, 8:9, :])  # kpat first
        nc.sync.dma_start(lutrj[:, 0:3, :], lut_d[:, 0:3, :])
        nc.scalar.dma_start(lutrj[:, 3:6, :], lut_d[:, 3:6, :])
        nc.scalar.dma_start(lutrj[:, 6:8, :], lut_d[:, 6:8, :])

        T21, T43, T65, T87, T109, T1211, T1413 = 15, 16, 17, 18, 19, 20, 21

        tt = nc.vector.tensor_tensor
        tsc = nc.vector.tensor_scalar
        stt = nc.vector.scalar_tensor_tensor
        cp = nc.vector.copy_predicated
        cpy = nc.vector.tensor_copy

        t0 = 0
        off = 0
        for g, Gg in enumerate(SIZES):
            W = Gg * 64  # free width of this group's descent ops
            ch_fl = chpool.tile([128, DEPTH * G * 64], F32, name="ch", tag="ch")
            chg = ch_fl[:, 0:DEPTH * W].rearrange("p (d w) -> p d w", d=DEPTH)
            src = x_d[:, off:off + DEPTH * W].rearrange(
                "p (d w) -> p d w", d=DEPTH
            )
            uh = DEPTH // 2
            nc.sync.dma_start(chg[:, 0:uh, :], src[:, 0:uh, :])
            nc.scalar.dma_start(chg[:, uh:DEPTH, :], src[:, uh:DEPTH, :])

            tmp = {
                n: tmppool.tile([128, G * 64], F32, name=n, tag=n)[:, 0:W]
                for n in ["b0", "b1", "b2", "b3", "ts0", "ts1", "ts2", "ts3"]
            }
            b0, b1, b2, b3 = tmp["b0"], tmp["b1"], tmp["b2"], tmp["b3"]
            ts0, ts1, ts2, ts3 = tmp["ts0"], tmp["ts1"], tmp["ts2"], tmp["ts3"]
            b0i = tmppool.tile([128, G * 64], I8, name="b0i", tag="b0i")[:, 0:W]
            b1i = tmppool.tile([128, G * 64], I8, name="b1i", tag="b1i")[:, 0:W]
            bkt = bkpool.tile([128, G * 64], BF16, name="bkt", tag="bkt")
            bkw = bkt[:, 0:W]

            def xd(d, chg=chg):
                return chg[:, d, :]

            # depth 0
            tsc(b0, xd(0), tcol(0), None, ALU.is_gt)
            cpy(b0i, b0)
            # depth 1: thr = t1 + b0*(t2-t1), fused
            tsc(ts0, b0, tcol(T21), tcol(1), ALU.mult, ALU.add)
            tt(b1, xd(1), ts0, ALU.is_gt)
            cpy(b1i, b1)
            # depth 2
            tsc(ts0, b1, tcol(T43), tcol(3), ALU.mult, ALU.add)
            tsc(ts1, b1, tcol(T65), tcol(5), ALU.mult, ALU.add)
            cp(ts0, b0i, ts1)
            tt(b2, xd(2), ts0, ALU.is_gt)
            # depth 3
            tsc(ts0, b2, tcol(T87), tcol(7), ALU.mult, ALU.add)
            tsc(ts1, b2, tcol(T109), tcol(9), ALU.mult, ALU.add)
            tsc(ts2, b2, tcol(T1211), tcol(11), ALU.mult, ALU.add)
            tsc(ts3, b2, tcol(T1413), tcol(13), ALU.mult, ALU.add)
            cp(ts0, b1i, ts1)
            cp(ts2, b1i, ts3)
            cp(ts0, b0i, ts2)
            tt(b3, xd(3), ts0, ALU.is_gt)
            # bucket = 8*b0 + 4*b1 + 2*b2 + b3 (bf16 exact)
            stt(ts1, b0, 2.0, b1, ALU.mult, ALU.add)
            stt(ts2, ts1, 2.0, b2, ALU.mult, ALU.add)
            stt(bkw, ts2, 2.0, b3, ALU.mult, ALU.add)

            # transpose bucket to n-major: bktN[p=n%128, mid, (h,c)]
            nmid = W // 128  # = Gg // 2
            bktN = bnpool.tile([128, G // 2, 128], BF16, name="bktN", tag="bktN")
            nc.scalar.dma_start_transpose(bktN[:, 0:nmid, :], bkw)

            osb = opool.tile([128, G, M], BF16, name="osb", tag="osb")
            kp4 = kpat.rearrange("p (j k c) -> p j k c", j=8, k=K)
            for t8 in range(Gg):
                h = t8 // nmid
                mid = t8 % nmid
                E = epool.tile([128, C * K], BF16, name="E", tag="E")
                in0 = bktN[:, mid:mid + 1, h * 64:(h + 1) * 64].rearrange(
                    "p one (j c) -> p j one c", j=8
                ).broadcast_to([128, 8, K, 8])
                tt(
                    E[:].rearrange("p (j k c) -> p j k c", j=8, k=K),
                    in0,
                    kp4,
                    ALU.is_equal,
                )
                et = etpool.tile([128, 8, 128], BF16, name="et", tag="et")
                nc.scalar.dma_start_transpose(et[:], E[:])

                ps = [
                    pspool.tile([128, 512], F32, name=f"ps{mc}", tag=f"ps{mc}")
                    for mc in range(2)
                ]
                for j in range(8):
                    lhsT = et[:, j, :]
                    for mc in range(2):
                        nc.tensor.matmul(
                            ps[mc][:], lhsT, lutT[:, j, mc * 512:(mc + 1) * 512],
                            start=(j == 0), stop=(j == 7),
                        )
                nc.scalar.activation(osb[:, t8, 0:512], ps[0][:], AFT.Copy)
                nc.scalar.activation(osb[:, t8, 512:1024], ps[1][:], AFT.Copy)

            oview = out_d[:].rearrange("(t p) m -> p t m", p=128)
            qs = [nc.gpsimd, nc.sync, nc.scalar]
            if g == NG - 1 and Gg > 2:
                half = Gg // 2
                qs[g % 3].dma_start(
                    oview[:, t0:t0 + half, :], osb[:, 0:half, :]
                )
                qs[(g + 1) % 3].dma_start(
                    oview[:, t0 + half:t0 + Gg, :], osb[:, half:Gg, :]
                )
            else:
                qs[g % 3].dma_start(oview[:, t0:t0 + Gg, :], osb[:, 0:Gg, :])
            t0 += Gg
            off += DEPTH * W
        es.close()
    nc.finalize()
    return nc


def _prep_inputs(inputMatrix, dims, thresholds, lut):
    x = np.asarray(inputMatrix, dtype=np.float32)
    dims_a = np.asarray(dims).ravel().astype(np.int64)
    thr = np.asarray(thresholds, dtype=np.float32).reshape(C, K - 1)
    lut = np.asarray(lut, dtype=np.float32)

    # chosen values [N, d, c]
    chv = x[:, dims_a.reshape(C, DEPTH)]          # [N, C, DEPTH]
    chv = np.ascontiguousarray(chv.transpose(0, 2, 1))  # [N, DEPTH, C]

    # group-blocked chosenT: per group section [d, (h: W-rows), nn]
    # cht[core][q=h*64+c, off + d*W + nn] = chv[r, d, c],
    #   r = core*4096 + t0*128 + h*W + nn, W = Gg*64
    cht = np.zeros((N_CORES, 128, 2 * NT * 128), dtype=np.float32)
    cv = chv.reshape(N_CORES, N_CORE, DEPTH, C)
    t0 = 0
    off = 0
    for Gg in SIZES:
        W = Gg * 64
        blk = cv[:, t0 * 128:t0 * 128 + 2 * W]          # [cores, 2W, d, c]
        blk = blk.reshape(N_CORES, 2, W, DEPTH, C)      # [cores, h, nn, d, c]
        blk = blk.transpose(0, 1, 4, 3, 2)              # [cores, h, c, d, nn]
        cht[:, :, off:off + DEPTH * W] = blk.reshape(
            N_CORES, 128, DEPTH * W
        )
        t0 += Gg
        off += DEPTH * W

    # thresholds per partition q = h*64 + c
    tcbm = np.zeros((NCONST, C), dtype=np.float32)
    for node in range(15):
        tcbm[node] = thr[:, node]
    pairs = [(2, 1), (4, 3), (6, 5), (8, 7), (10, 9), (12, 11), (14, 13)]
    for i, (hi, lo) in enumerate(pairs):
        tcbm[15 + i] = thr[:, hi] - thr[:, lo]
    cst = np.tile(tcbm.T, (2, 1)).astype(np.float32)    # [128, NCONST]

    lutTm = (
        lut.reshape(M, 8, 8, K).transpose(1, 3, 2, 0).reshape(C * K, M)
        .astype(ml_dtypes.bfloat16)
    )
    lutrj = np.zeros((128, 9, M), dtype=ml_dtypes.bfloat16)
    for j in range(8):
        lutrj[:, j, :] = lutTm[j * 128:(j + 1) * 128, :]
    e = np.arange(C * K)
    lutrj[:, 8, :] = ((e % 128) // 8).astype(ml_dtypes.bfloat16)[None]

    return cht, cst, lutrj


def _prep_all(inputs):
    cht, cst, lutrj = _prep_inputs(
        inputs["inputMatrix"], inputs["dims"], inputs["thresholds"], inputs["lut"]
    )
    nc = build_program()
    in_maps = [
        {"cht": cht[i], "cst": cst, "lutrj": lutrj}
        for i in range(N_CORES)
    ]
    return nc, in_maps


def kernel(inputMatrix, dims, thresholds, lut, selection_matrix=None,
           tree_des_mat=None):
    from concourse.bass_utils import run_bass_kernel_spmd

    nc, in_maps = _prep_all(
        {
            "inputMatrix": inputMatrix,
            "dims": dims,
            "thresholds": thresholds,
            "lut": lut,
        }
    )
    res = run_bass_kernel_spmd(nc, in_maps, list(range(N_CORES)))
    out = np.concatenate(
        [np.asarray(res.results[i]["out"]) for i in range(N_CORES)], axis=0
    )
    return out.astype(np.float32)
